# revision 2
# baseline (speedup 1.0000x reference)
"""Causal self-attention (B=2, T=2048, C=768, H=12) on 8 TRN2 NeuronCores.

Sharding: core c = (b = c // 4, head-group hg = c % 4 of 3 heads).
v2 layout, per core:
  - QKV column-parallel in 4 full 128-col weight chunks:
      chunk0 = [q_h0 | q_h1], chunk1 = [k_h0 | k_h1],
      chunk2 = [q_h2 | q_h2], chunk3 = [k_h2 | k_h2]  (h2 duplicated so its
      score matmuls can run as 64-contraction row-tiled pairs too).
  - Scores transposed [key, query], computed as row-tiled concurrent pairs
    (h0 rows 0-63 / h1 rows 64-127; h2 pairs two j-blocks via the dup).
  - exp on ScalarE over j-grouped multi-bank PSUM tiles (N up to 1024).
  - PV with v stationary ([128 keys, 64 v + 1 ones]) streaming the exp'd
    scores 512 wide -> y arrives transposed [d|den, query]; per-group
    partials are summed on DVE into SBUF, normalized by the broadcast
    reciprocal of the ones-column denominator.
  - Row-parallel out-projection straight from yT (no PE transposes), fp32
    partials summed host-side + bias.
"""

import os
import sys

import numpy as np
import ml_dtypes


def _ensure_paths():
    for p in ("/opt/trn_rl_repo", "/opt/pypackages"):
        if os.path.isdir(p) and p not in sys.path:
            sys.path.append(p)


_ensure_paths()

import concourse.bass as bass  # noqa: E402
import concourse.mybir as mybir  # noqa: E402
import concourse.tile as tile  # noqa: E402
from concourse import bacc  # noqa: E402
from concourse.bass_utils import run_bass_kernel_spmd  # noqa: E402

BF16 = ml_dtypes.bfloat16

B, T, C, H, D = 2, 2048, 768, 12, 64
G = 3                # heads per core
NT = T // 128        # 16 token tiles
KS = C // 128        # 6 contraction subtiles
QC = 4               # 512-query chunks

_cache: dict[bool, object] = {}
_last_in_maps = None


def _build(causal: bool):
    dt = mybir.dt
    nc = bacc.Bacc("TRN2", num_devices=8)

    xT_d = nc.dram_tensor("xT", [C, T], dt.bfloat16, kind="ExternalInput")
    wqkT_d = nc.dram_tensor("wqkT", [C, 512], dt.bfloat16, kind="ExternalInput")
    wvT_d = nc.dram_tensor("wvT", [C, G * D], dt.bfloat16, kind="ExternalInput")
    bqk_d = nc.dram_tensor("bqk", [128, 4], dt.float32, kind="ExternalInput")
    bv_d = nc.dram_tensor("bv", [128, G * D], dt.float32, kind="ExternalInput")
    wpT_d = nc.dram_tensor("wpT", [256, C], dt.bfloat16, kind="ExternalInput")
    maskT_d = nc.dram_tensor("maskT", [128, 128], dt.bfloat16, kind="ExternalInput")
    out_d = nc.dram_tensor("out", [T, C], dt.float32, kind="ExternalOutput")

    Exp = mybir.ActivationFunctionType.Exp
    ADD = mybir.AluOpType.add
    MUL = mybir.AluOpType.mult

    with tile.TileContext(nc) as tc:
        with tc.tile_pool(name="persist", bufs=1) as pp:
            xT_sb = pp.tile([128, KS, T], dt.bfloat16)
            wqkT_sb = pp.tile([128, KS, 512], dt.bfloat16)
            wvT_sb = pp.tile([128, KS, G * D], dt.bfloat16)
            wpT_sb = pp.tile([128, 2, C], dt.bfloat16)
            bqk_sb = pp.tile([128, 4], dt.float32)
            bv_sb = pp.tile([128, G * D], dt.float32)
            maskT_sb = pp.tile([128, 128], dt.bfloat16)
            qkT_sb = pp.tile([128, 4, T], dt.bfloat16)
            v_sb = pp.tile([128, NT, G, D + 1], dt.bfloat16)
            yT_sb = pp.tile([128, 2, T], dt.bfloat16)
            dum_sb = pp.tile([1, 4], dt.float32)

            # warm the ACT exp table while DMAs/QKV run
            nc.gpsimd.memset(dum_sb[:], 0.0)
            nc.scalar.activation(dum_sb[:], dum_sb[:], Exp)

            for s in range(KS):
                nc.sync.dma_start(
                    xT_sb[:, s, :], xT_d.ap()[s * 128 : (s + 1) * 128, :]
                )
                nc.sync.dma_start(
                    wqkT_sb[:, s, :], wqkT_d.ap()[s * 128 : (s + 1) * 128, :]
                )
            nc.sync.dma_start(
                wvT_sb[:], wvT_d.ap().rearrange("(s p) f -> p s f", p=128)
            )
            nc.sync.dma_start(
                wpT_sb[:], wpT_d.ap().rearrange("(s p) o -> p s o", p=128)
            )
            nc.sync.dma_start(bqk_sb[:], bqk_d.ap())
            nc.sync.dma_start(bv_sb[:], bv_d.ap())
            nc.sync.dma_start(maskT_sb[:], maskT_d.ap())
            nc.gpsimd.memset(v_sb[:, :, :, D : D + 1], 1.0)

            # ---- Phase 1a: q/k projection -> qkT_sb [f, t] (bf16, +bias) ----
            with tc.tile_pool(name="ps_qk", bufs=3, space="PSUM") as qkps:
                for ci in range(4):
                    for tch in range(4):
                        ps = qkps.tile([128, 512], dt.float32)
                        for s in range(KS):
                            nc.tensor.matmul(
                                ps[:],
                                wqkT_sb[:, s, ci * 128 : (ci + 1) * 128],
                                xT_sb[:, s, tch * 512 : (tch + 1) * 512],
                                start=(s == 0),
                                stop=(s == KS - 1),
                            )
                        nc.vector.tensor_scalar_add(
                            qkT_sb[:, ci, tch * 512 : (tch + 1) * 512],
                            ps[:],
                            bqk_sb[:, ci : ci + 1],
                        )

            # ---- Phase 1b: v projection -> v_sb [t, g, d|1] (bf16, +bias) ----
            with tc.tile_pool(name="ps_v", bufs=2, space="PSUM") as vps:
                for ti in range(NT):
                    ps = vps.tile([128, G * D], dt.float32)
                    for s in range(KS):
                        nc.tensor.matmul(
                            ps[:],
                            xT_sb[:, s, ti * 128 : (ti + 1) * 128],
                            wvT_sb[:, s, :],
                            start=(s == 0),
                            stop=(s == KS - 1),
                        )
                    for h in range(G):
                        nc.vector.tensor_tensor(
                            v_sb[:, ti, h, 0:D],
                            ps[:, h * D : (h + 1) * D],
                            bv_sb[:, h * D : (h + 1) * D],
                            ADD,
                        )

            # ---- Phase 2: attention + out-projection ----
            # 4 psum slots of 2 banks each; score groups rotate through them,
            # PV partials and the out-proj psum alias into the same slots.
            with (
                tc.tile_pool(name="ps2", bufs=1, space="PSUM") as ps2,
                tc.tile_pool(name="ptp", bufs=4) as ptp,
                tc.tile_pool(name="yap", bufs=3) as yap,
                tc.tile_pool(name="rcp", bufs=2) as rcp,
                tc.tile_pool(name="shp", bufs=2) as shp,
                tc.tile_pool(name="obp", bufs=2) as obp,
            ):
                SLOT_TAGS = ["A0", "A1", "B0", "B1"]

                def slot_tile(tag):
                    t_ = ps2.tile(
                        [128, 1024], dt.float32, tag=tag, name=f"slot_{tag}"
                    )
                    return t_

                def make_groups(qc):
                    """Groups of (j, w, off) score blocks packed into one
                    <=1024-col slot; off = col offset in slot, w = computed
                    query width; queries covered = [512-w, 512) of the chunk."""
                    if not causal:
                        return [
                            [(2 * g, 512, 0), (2 * g + 1, 512, 512)]
                            for g in range(NT // 2)
                        ]
                    groups = [
                        [(2 * g, 512, 0), (2 * g + 1, 512, 512)]
                        for g in range(2 * qc)
                    ]
                    jb = 4 * qc
                    groups.append([(jb + 0, 512, 0), (jb + 1, 384, 512)])
                    groups.append([(jb + 2, 256, 0), (jb + 3, 128, 256)])
                    return groups

                def attn(qc, heads):
                    """heads = (0,1) pair-tiled, or (2,) with dup row-pairs."""
                    q0 = qc * 512
                    pair = len(heads) == 2
                    groups = make_groups(qc)
                    yaccs = {}
                    for h in heads:
                        ya = yap.tile([128, 512], dt.float32, name=f"yacc{h}")
                        yaccs[h] = ya
                    for gi, grp in enumerate(groups):
                        n = grp[-1][2] + grp[-1][1]  # packed width
                        lo = grp[0][2] + 512 - grp[0][1]  # first valid col
                        if pair:
                            slots = [
                                slot_tile(SLOT_TAGS[(gi % 2) * 2 + ln])
                                for ln in range(2)
                            ]
                        else:
                            slots = [slot_tile(SLOT_TAGS[gi % 4])]
                        # scores
                        for bi, (j, w, off) in enumerate(grp):
                            for ln, h in enumerate(heads):
                                if pair:
                                    p0, sp = 64 * ln, slots[ln]
                                    kc, qc_ = 1, 0
                                else:
                                    sp = slots[0]
                                    kc, qc_ = 3, 2
                                    # dup row-pair except same-bank conflict
                                    p0 = 64 * (bi % 2)
                                    if off // 512 == (grp[0][2] // 512):
                                        p0 = 0  # same bank: keep sequential
                                nc.tensor.matmul(
                                    sp[:, off : off + w],
                                    qkT_sb[p0 : p0 + 64, kc, j * 128 : (j + 1) * 128],
                                    qkT_sb[p0 : p0 + 64, qc_, q0 + 512 - w : q0 + 512],
                                    start=True,
                                    stop=True,
                                )
                        # exp (one per lane over the packed region)
                        pts = []
                        for ln, h in enumerate(heads):
                            pt = ptp.tile([128, 1024], dt.bfloat16, name=f"pt{ln}")
                            nc.scalar.activation(
                                pt[:, 0:n], slots[ln][:, 0:n], Exp, scale=0.125
                            )
                            pts.append(pt)
                        # causal diagonal masks (band groups only)
                        if causal and grp[0][0] >= 4 * qc:
                            for (j, w, off) in grp:
                                for ln in range(len(heads)):
                                    nc.vector.tensor_tensor(
                                        pts[ln][:, off : off + 128],
                                        pts[ln][:, off : off + 128],
                                        maskT_sb[:],
                                        MUL,
                                    )
                        # PV: v stationary, stream exp'd scores 512-wide
                        for ln, h in enumerate(heads):
                            for bi, (j, w, off) in enumerate(grp):
                                nc.tensor.matmul(
                                    slots[ln][0:65, 512 - w : 512],
                                    v_sb[:, j, h, :],
                                    pts[ln][:, off : off + w],
                                    start=(bi == 0),
                                    stop=(bi == len(grp) - 1),
                                    skip_group_check=True,
                                )
                        # accumulate partials on DVE into SBUF
                        for ln, h in enumerate(heads):
                            ya = yaccs[h]
                            w0 = grp[0][1]  # first block is widest
                            if gi == 0:
                                nc.vector.tensor_copy(
                                    ya[0:65, 512 - w0 : 512],
                                    slots[ln][0:65, 512 - w0 : 512],
                                )
                            else:
                                nc.vector.tensor_tensor(
                                    ya[0:65, 512 - w0 : 512],
                                    slots[ln][0:65, 512 - w0 : 512],
                                    ya[0:65, 512 - w0 : 512],
                                    ADD,
                                )
                    # normalize: yT = y * (1/den), den = ones-column row 64
                    for h in heads:
                        ya = yaccs[h]
                        rr = rcp.tile([1, 512], dt.float32, name="rr")
                        rb = rcp.tile([64, 512], dt.float32, name="rb")
                        nc.vector.reciprocal(rr[0:1, :], ya[64:65, :])
                        nc.gpsimd.partition_broadcast(rb[0:64, :], rr[0:1, :])
                        if h == 1:
                            yt = shp.tile([64, 512], dt.bfloat16, name="yt")
                            nc.vector.tensor_tensor(
                                yt[0:64, :], ya[0:64, :], rb[0:64, :], MUL
                            )
                            nc.sync.dma_start(
                                yT_sb[64:128, 0, q0 : q0 + 512], yt[0:64, :]
                            )
                        else:
                            ch = 0 if h == 0 else 1
                            nc.vector.tensor_tensor(
                                yT_sb[0:64, ch, q0 : q0 + 512],
                                ya[0:64, :],
                                rb[0:64, :],
                                MUL,
                            )

                for qc in range(QC):
                    attn(qc, (0, 1))
                    attn(qc, (2,))
                    # out-projection for this query chunk
                    for t_in in range(4):
                        qt = 4 * qc + t_in
                        po = ps2.tile(
                            [128, C], dt.float32, tag=SLOT_TAGS[qt % 4], name="po"
                        )
                        for ch in range(2):
                            c0 = ch * 512
                            cw = 512 if ch == 0 else 256
                            nc.tensor.matmul(
                                po[:, c0 : c0 + cw],
                                yT_sb[:, 0, qt * 128 : (qt + 1) * 128],
                                wpT_sb[:, 0, c0 : c0 + cw],
                                start=True,
                                stop=False,
                                skip_group_check=True,
                            )
                            nc.tensor.matmul(
                                po[:, c0 : c0 + cw],
                                yT_sb[0:64, 1, qt * 128 : (qt + 1) * 128],
                                wpT_sb[0:64, 1, c0 : c0 + cw],
                                start=False,
                                stop=True,
                                skip_group_check=True,
                            )
                        ob = obp.tile([128, C], dt.float32, name="ob")
                        nc.vector.tensor_copy(ob[:], po[:])
                        nc.sync.dma_start(
                            out_d.ap()[qt * 128 : (qt + 1) * 128, :], ob[:]
                        )

    nc.compile()
    return nc


def _prep_in_maps(x, Wqkv, bqkv, Wproj):
    in_maps = []
    for c in range(8):
        b, hg = c // 4, c % 4
        r0 = 192 * hg
        xT = np.ascontiguousarray(x[b].T).astype(BF16)
        wq = Wqkv[r0 : r0 + 192]
        wk = Wqkv[768 + r0 : 768 + r0 + 192]
        wqk = np.concatenate(
            [wq[0:128], wk[0:128], wq[128:192], wq[128:192], wk[128:192], wk[128:192]],
            axis=0,
        )  # [512, 768]
        wqkT = np.ascontiguousarray(wqk.T).astype(BF16)
        wvT = np.ascontiguousarray(Wqkv[1536 + r0 : 1536 + r0 + 192].T).astype(BF16)
        bq = bqkv[r0 : r0 + 192]
        bk = bqkv[768 + r0 : 768 + r0 + 192]
        bqk = np.stack(
            [
                bq[0:128],
                bk[0:128],
                np.concatenate([bq[128:192], bq[128:192]]),
                np.concatenate([bk[128:192], bk[128:192]]),
            ],
            axis=1,
        ).astype(np.float32)  # [128, 4]
        bv = np.tile(
            bqkv[1536 + r0 : 1536 + r0 + 192].astype(np.float32)[None, :], (128, 1)
        )
        wp = np.zeros((256, 768), dtype=BF16)
        wp[0:192] = Wproj[:, r0 : r0 + 192].T.astype(BF16)
        maskT = np.triu(np.ones((128, 128), dtype=np.float32)).astype(BF16)
        in_maps.append(
            {
                "xT": xT,
                "wqkT": wqkT,
                "wvT": wvT,
                "bqk": np.ascontiguousarray(bqk),
                "bv": bv,
                "wpT": wp,
                "maskT": maskT,
            }
        )
    return in_maps


def kernel(x, Wqkv, bqkv, Wproj, bproj, is_causal):
    global _last_in_maps
    x = np.asarray(x, dtype=np.float32)
    Wqkv = np.asarray(Wqkv, dtype=np.float32)
    bqkv = np.asarray(bqkv, dtype=np.float32)
    Wproj = np.asarray(Wproj, dtype=np.float32)
    bproj = np.asarray(bproj, dtype=np.float32)
    causal = bool(int(np.asarray(is_causal)))

    if causal not in _cache:
        _cache[causal] = _build(causal)
    nc = _cache[causal]

    in_maps = _prep_in_maps(x, Wqkv, bqkv, Wproj)
    _last_in_maps = in_maps
    res = run_bass_kernel_spmd(nc, in_maps, core_ids=list(range(8)))

    out = np.empty((B, T, C), dtype=np.float32)
    for b in range(B):
        acc = res.results[4 * b]["out"].copy()
        for k in range(1, 4):
            acc += res.results[4 * b + k]["out"]
        out[b] = acc + bproj[None, :]
    return out
